# revision 10
# baseline (speedup 1.0000x reference)
"""MoE router kernel for Trainium2 (8 NeuronCores, token-dim sharding).

Computes, for full inputs x[16384,2048], w_gate[2048,64], noise[16384,64]:
  scores  = x @ w_gate
  probs   = softmax(scores + noise)       (never materialized -- cancels)
  top-2   values/indices per token
  gates   = renormalized top-2 probs  = sigmoid(+/-(s1 - s2))
  combine = gates scattered into [T, E]
  expert_activation = per-expert selection counts

Sharding: token dim T split 8 ways (2048 tokens/core); w_gate replicated;
expert_activation partial counts summed on host after gather.

Device kernel per 128-token chunk:
  - PE transposes x tiles (fp32, via identity matmul) -> xT in PSUM
  - DVE/ACT copy xT PSUM->SBUF
  - PE matmul accumulates scores[t,e] = sum_d xT[d,t]^T @ w[d,e]
  - DVE adds noise, max8 + max_index give top-2 vals/idx
  - gates via sigmoid(+/-(v1-v2)) on ACT (softmax denominator cancels)
  - combine via compare-masks against v1/v2 (no scatter needed)
  - counts via ones^T @ (s >= v2) matmul accumulated in PSUM
"""

from contextlib import ExitStack

import numpy as np

import concourse.bass as bass
import concourse.mybir as mybir
import concourse.tile as tile
from concourse import bacc
from concourse.bass_utils import run_bass_kernel_spmd
from concourse.masks import make_identity

N_CORES = 8
T, D, E = 16384, 2048, 64
TS = T // N_CORES          # tokens per core
P = 128                    # partitions
N_TC = TS // P             # token chunks per core
N_DC = D // P              # contraction (d) chunks
DB = 4                     # d-chunks batched per PSUM transpose tile

FP32 = mybir.dt.float32
U32 = mybir.dt.uint32


def _build_bass():
    nc = bacc.Bacc(trn_type="TRN2", debug=False, num_devices=N_CORES)

    x_d = nc.dram_tensor("x", [TS, D], FP32, kind="ExternalInput").ap()
    w_d = nc.dram_tensor("w", [D, E], FP32, kind="ExternalInput").ap()
    noise_d = nc.dram_tensor("noise", [TS, E], FP32, kind="ExternalInput").ap()

    combine_d = nc.dram_tensor("combine", [TS, E], FP32, kind="ExternalOutput").ap()
    # misc packs [g1, g2, idx1(bits), idx2(bits)] per token
    misc_d = nc.dram_tensor("misc", [TS, 4], FP32, kind="ExternalOutput").ap()
    cnt_d = nc.dram_tensor("cnt", [1, E], FP32, kind="ExternalOutput").ap()

    with tile.TileContext(nc) as tc, ExitStack() as ctx:
        consts = ctx.enter_context(tc.tile_pool(name="consts", bufs=1))
        xin = ctx.enter_context(tc.tile_pool(name="xin", bufs=3))
        xtp = ctx.enter_context(tc.tile_pool(name="xtp", bufs=4))
        small = ctx.enter_context(tc.tile_pool(name="small", bufs=3))
        psum_t = ctx.enter_context(tc.tile_pool(name="psum_t", bufs=4, space="PSUM"))
        psum_s = ctx.enter_context(tc.tile_pool(name="psum_s", bufs=2, space="PSUM"))
        psum_c = ctx.enter_context(tc.tile_pool(name="psum_c", bufs=1, space="PSUM"))

        identity = consts.tile([P, P], FP32)
        make_identity(nc, identity)
        ones = consts.tile([P, 1], FP32)
        nc.vector.memset(ones, 1.0)

        # w as [128 d-in-chunk, 16 d-chunks, 64 experts]
        w_sb = consts.tile([P, N_DC, E], FP32)
        nc.sync.dma_start(out=w_sb, in_=w_d.rearrange("(c p) e -> p c e", p=P))

        cnt_ps = psum_c.tile([1, E], FP32)

        # PE wait-absorber: the fused fp32 matmul/transpose (LW struct) has a
        # single sync-wait slot, so absorb the w_sb DMA wait into a tiny
        # standalone bf16 LDWEIGHTS (writes no tile -> never carries WAW deps;
        # each later fp32 matmul self-loads its weights, so the garbage load is
        # harmless).
        nc.tensor.ldweights(w_sb[:, 0, 0:1].bitcast(mybir.dt.bfloat16))

        for tcv in range(N_TC):
            rows = slice(tcv * P, (tcv + 1) * P)
            x_tile = xin.tile([P, D], FP32)
            nc.sync.dma_start(out=x_tile, in_=x_d[rows, :])
            noise_t = small.tile([P, E], FP32)
            nc.sync.dma_start(out=noise_t, in_=noise_d[rows, :])

            # absorb the x_tile DMA wait so the first transpose of the chunk
            # only carries its PSUM slot-reuse wait
            nc.tensor.ldweights(x_tile[:, 0:1].bitcast(mybir.dt.bfloat16))

            score_ps = psum_s.tile([P, E], FP32)
            for db in range(N_DC // DB):
                xt_ps = psum_t.tile([P, DB * P], FP32)
                for j in range(DB):
                    dc = db * DB + j
                    nc.tensor.transpose(
                        xt_ps[:, j * P : (j + 1) * P],
                        x_tile[:, dc * P : (dc + 1) * P],
                        identity,
                    )
                xt_sb = xtp.tile([P, DB * P], FP32)
                if db % 2 == 0:
                    nc.vector.tensor_copy(xt_sb, xt_ps)
                else:
                    nc.scalar.copy(xt_sb, xt_ps)
                for j in range(DB):
                    dc = db * DB + j
                    nc.tensor.matmul(
                        score_ps,
                        lhsT=xt_sb[:, j * P : (j + 1) * P],
                        rhs=w_sb[:, dc, :],
                        start=(dc == 0),
                        stop=(dc == N_DC - 1),
                    )

            # s = scores + noise
            s_t = small.tile([P, E], FP32)
            nc.vector.tensor_add(s_t, score_ps, noise_t)

            top8 = small.tile([P, 8], FP32)
            nc.vector.max(out=top8, in_=s_t)
            idx8 = small.tile([P, 8], U32)
            nc.vector.max_index(out=idx8, in_max=top8, in_values=s_t)

            dv = small.tile([P, 1], FP32)
            nc.vector.tensor_sub(dv, top8[:, 0:1], top8[:, 1:2])

            misc_t = small.tile([P, 4], FP32)
            nc.scalar.activation(
                misc_t[:, 0:1], dv, mybir.ActivationFunctionType.Sigmoid,
                bias=0.0, scale=1.0,
            )
            nc.scalar.activation(
                misc_t[:, 1:2], dv, mybir.ActivationFunctionType.Sigmoid,
                bias=0.0, scale=-1.0,
            )
            nc.vector.tensor_copy(misc_t[:, 2:4].bitcast(U32), idx8[:, 0:2])

            cmp2 = small.tile([P, E], FP32)
            nc.vector.tensor_scalar(
                cmp2, s_t, top8[:, 1:2], None, op0=mybir.AluOpType.is_ge
            )
            cmp1 = small.tile([P, E], U32)
            nc.vector.tensor_scalar(
                cmp1, s_t, top8[:, 0:1], None, op0=mybir.AluOpType.is_ge
            )

            comb_t = small.tile([P, E], FP32)
            nc.vector.tensor_scalar(
                comb_t, cmp2, misc_t[:, 1:2], None, op0=mybir.AluOpType.mult
            )
            nc.vector.copy_predicated(
                comb_t, cmp1, misc_t[:, 0:1].to_broadcast([P, E])
            )

            # expert activation counts: ones^T @ cmp2 accumulated over chunks
            nc.tensor.matmul(
                cnt_ps, lhsT=ones, rhs=cmp2,
                start=(tcv == 0), stop=(tcv == N_TC - 1),
                skip_group_check=True,
            )

            nc.sync.dma_start(out=combine_d[rows, :], in_=comb_t)
            nc.sync.dma_start(out=misc_d[rows, :], in_=misc_t)

        cnt_sb = consts.tile([1, E], FP32)
        nc.vector.tensor_copy(cnt_sb, cnt_ps)
        nc.sync.dma_start(out=cnt_d, in_=cnt_sb)

    nc.compile()
    return nc


_NC_CACHE = None


def kernel(x: np.ndarray, w_gate: np.ndarray, noise: np.ndarray, _trace=False):
    global _NC_CACHE
    x = np.ascontiguousarray(x, dtype=np.float32)
    w_gate = np.ascontiguousarray(w_gate, dtype=np.float32)
    noise = np.ascontiguousarray(noise, dtype=np.float32)

    in_maps = [
        {
            "x": np.ascontiguousarray(x[c * TS : (c + 1) * TS]),
            "w": w_gate,
            "noise": np.ascontiguousarray(noise[c * TS : (c + 1) * TS]),
        }
        for c in range(N_CORES)
    ]

    if _NC_CACHE is None:
        _NC_CACHE = _build_bass()

    res = run_bass_kernel_spmd(
        _NC_CACHE, in_maps, core_ids=list(range(N_CORES)), trace=_trace
    )

    combine = np.concatenate([res.results[c]["combine"] for c in range(N_CORES)], 0)
    misc = np.concatenate([res.results[c]["misc"] for c in range(N_CORES)], 0)
    gates = np.ascontiguousarray(misc[:, 0:2])
    topk_idx = np.ascontiguousarray(misc[:, 2:4]).view(np.int32)
    expert_activation = np.sum(
        [res.results[c]["cnt"][0] for c in range(N_CORES)], axis=0, dtype=np.float32
    )

    if _trace:
        kernel._last_results = res
    return combine, topk_idx, gates, expert_activation


# revision 14
# speedup vs baseline: 1.2494x; 1.2494x over previous
"""MoE router kernel for Trainium2 (8 NeuronCores, token-dim sharding).

Computes, for full inputs x[16384,2048], w_gate[2048,64], noise[16384,64]:
  scores  = x @ w_gate
  probs   = softmax(scores + noise)       (never materialized -- cancels)
  top-2   values/indices per token
  gates   = renormalized top-2 probs  = sigmoid(+/-(s1 - s2))
  combine = gates scattered into [T, E]
  expert_activation = per-expert selection counts

Sharding: token dim T split 8 ways (2048 tokens/core); w_gate replicated;
expert_activation partial counts summed on host after gather.

Device kernel per 128-token chunk:
  - PE transposes x tiles (fp32, via identity matmul) -> xT in PSUM
  - DVE/ACT copy xT PSUM->SBUF
  - PE matmul accumulates scores[t,e] = sum_d xT[d,t]^T @ w[d,e]
  - DVE adds noise, max8 + max_index give top-2 vals/idx
  - gates via sigmoid(+/-(v1-v2)) on ACT (softmax denominator cancels)
  - combine via compare-masks against v1/v2 (no scatter needed)
  - counts via ones^T @ (s >= v2) matmul accumulated in PSUM
"""

from contextlib import ExitStack

import numpy as np

import concourse.bass as bass
import concourse.mybir as mybir
import concourse.tile as tile
from concourse import bacc
from concourse.bass_utils import run_bass_kernel_spmd
from concourse.masks import make_identity

N_CORES = 8
T, D, E = 16384, 2048, 64
TS = T // N_CORES          # tokens per core
P = 128                    # partitions
N_TC = TS // P             # token chunks per core
N_DC = D // P              # contraction (d) chunks
DB = 4                     # d-chunks batched per PSUM transpose tile

FP32 = mybir.dt.float32
FP32R = mybir.dt.float32r
U32 = mybir.dt.uint32


def _round_fp32r(a: np.ndarray) -> np.ndarray:
    """Round fp32 to the TensorE float32r format (12 explicit mantissa bits,
    round-to-nearest-even), matching what the on-chip rounding copy would do."""
    bits = np.ascontiguousarray(a, dtype=np.float32).view(np.uint32)
    r = (bits + np.uint32(0x7FF) + ((bits >> np.uint32(12)) & np.uint32(1))) & np.uint32(
        0xFFFFF000
    )
    return r.view(np.float32)


def _build_bass():
    nc = bacc.Bacc(trn_type="TRN2", debug=False, num_devices=N_CORES)

    x_d = nc.dram_tensor("x", [TS, D], FP32R, kind="ExternalInput").ap()
    w_d = nc.dram_tensor("w", [D, E], FP32R, kind="ExternalInput").ap()
    noise_d = nc.dram_tensor("noise", [TS, E], FP32, kind="ExternalInput").ap()

    combine_d = nc.dram_tensor("combine", [TS, E], FP32, kind="ExternalOutput").ap()
    # misc packs [g1, g2, idx1(bits), idx2(bits)] per token
    misc_d = nc.dram_tensor("misc", [TS, 4], FP32, kind="ExternalOutput").ap()
    cnt_d = nc.dram_tensor("cnt", [1, E], FP32, kind="ExternalOutput").ap()

    with tile.TileContext(nc) as tc, ExitStack() as ctx:
        consts = ctx.enter_context(tc.tile_pool(name="consts", bufs=1))
        xin = ctx.enter_context(tc.tile_pool(name="xin", bufs=3))
        xtp = ctx.enter_context(tc.tile_pool(name="xtp", bufs=4))
        small = ctx.enter_context(tc.tile_pool(name="small", bufs=3))
        psum_t = ctx.enter_context(tc.tile_pool(name="psum_t", bufs=4, space="PSUM"))
        psum_s = ctx.enter_context(tc.tile_pool(name="psum_s", bufs=2, space="PSUM"))
        psum_c = ctx.enter_context(tc.tile_pool(name="psum_c", bufs=1, space="PSUM"))

        identity_f = consts.tile([P, P], FP32)
        make_identity(nc, identity_f)
        identity = consts.tile([P, P], FP32R)
        nc.vector.tensor_copy(identity, identity_f)
        ones = consts.tile([P, 1], FP32)
        nc.vector.memset(ones, 1.0)

        # w as [128 d-in-chunk, 16 d-chunks, 64 experts]
        w_sb = consts.tile([P, N_DC, E], FP32R)
        nc.sync.dma_start(out=w_sb, in_=w_d.rearrange("(c p) e -> p c e", p=P))

        cnt_ps = psum_c.tile([1, E], FP32)

        # PE wait-absorber: the fused fp32 matmul/transpose (LW struct) has a
        # single sync-wait slot, so absorb the w_sb DMA wait into a tiny
        # standalone bf16 LDWEIGHTS (writes no tile -> never carries WAW deps;
        # each later fp32 matmul self-loads its weights, so the garbage load is
        # harmless).
        nc.tensor.ldweights(w_sb[:, 0, 0:1].bitcast(mybir.dt.bfloat16))

        for tcv in range(N_TC):
            rows = slice(tcv * P, (tcv + 1) * P)
            x_tile = xin.tile([P, D], FP32R)
            nc.sync.dma_start(out=x_tile, in_=x_d[rows, :])
            noise_t = small.tile([P, E], FP32)
            nc.sync.dma_start(out=noise_t, in_=noise_d[rows, :])

            # absorb the x_tile DMA wait so the first transpose of the chunk
            # only carries its PSUM slot-reuse wait
            nc.tensor.ldweights(x_tile[:, 0:1].bitcast(mybir.dt.bfloat16))

            score_ps = psum_s.tile([P, E], FP32)
            for db in range(N_DC // DB):
                xt_ps = psum_t.tile([P, DB * P], FP32R)
                for j in range(DB):
                    dc = db * DB + j
                    nc.tensor.transpose(
                        xt_ps[:, j * P : (j + 1) * P],
                        x_tile[:, dc * P : (dc + 1) * P],
                        identity,
                    )
                xt_sb = xtp.tile([P, DB * P], FP32R)
                if db % 2 == 0:
                    nc.vector.tensor_copy(xt_sb, xt_ps)
                else:
                    nc.scalar.copy(xt_sb, xt_ps)
                for j in range(DB):
                    dc = db * DB + j
                    nc.tensor.matmul(
                        score_ps,
                        lhsT=xt_sb[:, j * P : (j + 1) * P],
                        rhs=w_sb[:, dc, :],
                        start=(dc == 0),
                        stop=(dc == N_DC - 1),
                    )

            # s = scores + noise
            s_t = small.tile([P, E], FP32)
            nc.vector.tensor_add(s_t, score_ps, noise_t)

            top8 = small.tile([P, 8], FP32)
            nc.vector.max(out=top8, in_=s_t)
            idx8 = small.tile([P, 8], U32)
            nc.vector.max_index(out=idx8, in_max=top8, in_values=s_t)

            dv = small.tile([P, 1], FP32)
            nc.vector.tensor_sub(dv, top8[:, 0:1], top8[:, 1:2])

            misc_t = small.tile([P, 4], FP32)
            nc.scalar.activation(
                misc_t[:, 0:1], dv, mybir.ActivationFunctionType.Sigmoid,
                bias=0.0, scale=1.0,
            )
            nc.scalar.activation(
                misc_t[:, 1:2], dv, mybir.ActivationFunctionType.Sigmoid,
                bias=0.0, scale=-1.0,
            )
            nc.vector.tensor_copy(misc_t[:, 2:4].bitcast(U32), idx8[:, 0:2])

            cmp2 = small.tile([P, E], FP32)
            nc.vector.tensor_scalar(
                cmp2, s_t, top8[:, 1:2], None, op0=mybir.AluOpType.is_ge
            )
            cmp1 = small.tile([P, E], U32)
            nc.vector.tensor_scalar(
                cmp1, s_t, top8[:, 0:1], None, op0=mybir.AluOpType.is_ge
            )

            comb_t = small.tile([P, E], FP32)
            nc.vector.tensor_scalar(
                comb_t, cmp2, misc_t[:, 1:2], None, op0=mybir.AluOpType.mult
            )
            nc.vector.copy_predicated(
                comb_t, cmp1, misc_t[:, 0:1].to_broadcast([P, E])
            )

            # expert activation counts: ones^T @ cmp2 accumulated over chunks
            nc.tensor.matmul(
                cnt_ps, lhsT=ones, rhs=cmp2,
                start=(tcv == 0), stop=(tcv == N_TC - 1),
                skip_group_check=True,
            )

            nc.sync.dma_start(out=combine_d[rows, :], in_=comb_t)
            nc.sync.dma_start(out=misc_d[rows, :], in_=misc_t)

        cnt_sb = consts.tile([1, E], FP32)
        nc.vector.tensor_copy(cnt_sb, cnt_ps)
        nc.sync.dma_start(out=cnt_d, in_=cnt_sb)

    nc.compile()
    return nc


_NC_CACHE = None


def kernel(x: np.ndarray, w_gate: np.ndarray, noise: np.ndarray, _trace=False):
    global _NC_CACHE
    x = _round_fp32r(x)
    w_gate = _round_fp32r(w_gate)
    noise = np.ascontiguousarray(noise, dtype=np.float32)

    in_maps = [
        {
            "x": np.ascontiguousarray(x[c * TS : (c + 1) * TS]),
            "w": w_gate,
            "noise": np.ascontiguousarray(noise[c * TS : (c + 1) * TS]),
        }
        for c in range(N_CORES)
    ]

    if _NC_CACHE is None:
        _NC_CACHE = _build_bass()

    res = run_bass_kernel_spmd(
        _NC_CACHE, in_maps, core_ids=list(range(N_CORES)), trace=_trace
    )

    combine = np.concatenate([res.results[c]["combine"] for c in range(N_CORES)], 0)
    misc = np.concatenate([res.results[c]["misc"] for c in range(N_CORES)], 0)
    gates = np.ascontiguousarray(misc[:, 0:2])
    topk_idx = np.ascontiguousarray(misc[:, 2:4]).view(np.int32)
    expert_activation = np.sum(
        [res.results[c]["cnt"][0] for c in range(N_CORES)], axis=0, dtype=np.float32
    )

    if _trace:
        kernel._last_results = res
    return combine, topk_idx, gates, expert_activation
